# revision 14
# baseline (speedup 1.0000x reference)
"""Trainium2 Bass kernel for nn_DepDagLSTMPool (gnn_message_passing).

Sharding: nodes partitioned across 8 cores (12500/core). Edges assigned to the
core owning their softmax-segment (query) node. Per-core phases:
  A: LayerNorm + Q projections for owned nodes; AllGather x.
  B: edge attention — uniform 128-edge tiles in <=128-node windows; K/V computed
     per-edge from gathered x rows; segment softmax via one-hot matmuls
     accumulated in PSUM; normalized aggregate scatter-written per window.
  C: Wo matmuls + type gates + W_dag + tanh -> pooled; AllGather pooled.
  D: pool segment-max via layered indirect gathers; conj set gather; output.
"""
import os
import sys
sys.path.insert(0, "/opt/trn_rl_repo")
os.environ.setdefault("NEURON_SCRATCHPAD_PAGE_SIZE", "1024")
import numpy as np

import concourse.bass as bass
import concourse.bacc as bacc
import concourse.tile as tile
from concourse import mybir
from concourse import bass_utils
from concourse.masks import make_identity

F32 = mybir.dt.float32
F32R = mybir.dt.float32r
I32 = mybir.dt.int32

NC = 8
N = 100000
D = 256
ATT = 512
H = 8
HD = 64
E = 250000
P = 8192
PC = 1024
MP = 131072
MC = 16384
TYPE_CT = 64
TS = 64
DC = D + ATT          # 768
NPC = N // NC         # 12500 nodes per core
SCRATCH = NPC         # scratch row index in agg buffers
PSEG = P // NC        # 1024 prem segs per core
CSEG = PC // NC       # 128 conj segs per core
WIN_NODES = 128
WIN_EDGES = 384       # 3 tiles of 128 edges per window


# ---------------------------------------------------------------- host prep

def _pack_edges(src_g, tgt_g, lo):
    """Edges whose (local) src is owned: window packing.
    Returns (em_i32 [T,128,2], em_seg [T,128,1], wrow [W,128,1]) lists per window."""
    s = src_g - lo
    sel = (s >= 0) & (s < NPC)
    s = s[sel].astype(np.int64)
    t = tgt_g[sel].astype(np.int64)
    order = np.argsort(s, kind="stable")
    s = s[order]; t = t[order]
    cnt = np.bincount(s, minlength=NPC)
    csum = np.concatenate([[0], np.cumsum(cnt)])
    assert cnt.max() <= WIN_EDGES, f"node degree {cnt.max()} exceeds window"

    wins = []
    n0 = 0
    while n0 < NPC:
        n1 = n0
        e = 0
        while n1 < NPC and (n1 - n0) < WIN_NODES:
            c = cnt[n1]
            if e + c > WIN_EDGES:
                break
            e += int(c); n1 += 1
        wins.append((n0, n1))
        n0 = n1

    em_i = []
    em_s = []
    for (n0, n1) in wins:
        e0, e1 = int(csum[n0]), int(csum[n1])
        ne = e1 - e0
        # int meta per window [128, 7]: (tgt,src) x 3 tiles + wrow
        mi = np.zeros((128, 7), np.int32)
        tgt = np.zeros((WIN_EDGES,), np.int32); tgt[:ne] = t[e0:e1]
        src = np.zeros((WIN_EDGES,), np.int32); src[:ne] = s[e0:e1]
        sg = np.full((WIN_EDGES,), -1.0, np.float32)
        sg[:ne] = (s[e0:e1] - n0).astype(np.float32)
        mi[:, 0:6:2] = tgt.reshape(3, 128).T
        mi[:, 1:6:2] = src.reshape(3, 128).T
        rw = np.full((128,), SCRATCH, np.int32)
        rw[: n1 - n0] = np.arange(n0, n1, dtype=np.int32)
        mi[:, 6] = rw
        em_i.append(mi)
        em_s.append(np.ascontiguousarray(sg.reshape(3, 128).T))  # [128, 3]
    return em_i, em_s


def _pad_windows(em_i, em_s, W):
    """Pad to exactly W windows with all-pad windows."""
    while len(em_i) < W:
        mi = np.zeros((128, 7), np.int32)
        mi[:, 6] = SCRATCH
        em_i.append(mi)
        em_s.append(np.full((128, 3), -1.0, np.float32))
    return (np.stack(em_i, 0), np.stack(em_s, 0))


def _pool_grid(nodes, segs, lo, nseg, L=None):
    """Layered entry grid for segment-max. nodes/segs: full arrays (segs sorted).
    Returns grid [L, nseg] int32 (global node ids), mask [nseg] f32, L."""
    i0, i1 = np.searchsorted(segs, lo), np.searchsorted(segs, lo + nseg)
    nd = nodes[i0:i1].astype(np.int64)
    sg = segs[i0:i1].astype(np.int64) - lo
    cnt = np.bincount(sg, minlength=nseg)
    csum = np.concatenate([[0], np.cumsum(cnt)])
    Lc = int(max(cnt.max(), 1))
    if L is None:
        L = Lc
    assert Lc <= L
    grid = np.zeros((L, nseg), np.int64)
    first = np.zeros(nseg, np.int64)
    ok = cnt > 0
    first[ok] = nd[csum[:-1][ok]]
    grid[:] = first[None, :]
    lay = np.arange(len(nd)) - csum[sg]
    grid[lay, sg] = nd
    mask = ok.astype(np.float32)
    return grid.astype(np.int32), mask, L


def preprocess(inputs):
    """Numpy index-only preprocessing -> per-core in_maps + program dims."""
    pc = np.asarray(inputs["pair_conj"])
    pp = np.asarray(inputs["pair_prem"])
    ppn = np.asarray(inputs["pool_prem_nodes"]); pps = np.asarray(inputs["pool_prem_seg"])
    pcn = np.asarray(inputs["pool_conj_nodes"]); pcs = np.asarray(inputs["pool_conj_seg"])
    p2c = np.asarray(inputs["prem2conj"])
    nt = np.asarray(inputs["node_types"])
    ns = np.asarray(inputs["node_states"])

    # per-core edge packing, both directions
    packs = []
    for c in range(NC):
        lo = c * NPC
        dp = _pack_edges(pp, pc, lo)   # dir p: src=prem, tgt=conj
        dc = _pack_edges(pc, pp, lo)   # dir c: src=conj, tgt=prem
        packs.append((dp, dc))
    W = max(max(len(d[0][0]), len(d[1][0])) for d in packs)

    # pool grids (uniform L across cores)
    Lp = max(_pool_grid(ppn, pps, c * PSEG, PSEG)[2] for c in range(NC))
    Lcj = max(_pool_grid(pcn, pcs, c * CSEG, CSEG)[2] for c in range(NC))

    # weights (shared across cores)
    def flat_qkv(wname):
        w = np.asarray(inputs[wname])          # [H, D, HD]
        return np.ascontiguousarray(w.transpose(1, 0, 2).reshape(D, ATT))

    shared = dict(
        ln_g=np.asarray(inputs["ln_g"]).reshape(1, D),
        ln_b=np.asarray(inputs["ln_b"]).reshape(1, D),
        wq_p=flat_qkv("Wq_p"), wk_p=flat_qkv("Wk_p"), wv_p=flat_qkv("Wv_p"),
        wq_c=flat_qkv("Wq_c"), wk_c=flat_qkv("Wk_c"), wv_c=flat_qkv("Wv_c"),
        wo_p=np.asarray(inputs["Wo_p"]), wo_c=np.asarray(inputs["Wo_c"]),
        wdag=np.asarray(inputs["W_dag"]), bdag=np.asarray(inputs["b_dag"]).reshape(1, DC),
        embp=np.asarray(inputs["emb_p"]), wpw=np.asarray(inputs["wp_w"]),
        wpb=np.asarray(inputs["wp_b"]).reshape(1, ATT),
        embc=np.asarray(inputs["emb_c"]), wcw=np.asarray(inputs["wc_w"]),
        wcb=np.asarray(inputs["wc_b"]).reshape(1, ATT),
    )
    shared = {k: np.ascontiguousarray(v, np.float32) for k, v in shared.items()}

    in_maps = []
    for c in range(NC):
        lo = c * NPC
        dp, dc = packs[c]
        emi_p, ems_p = _pad_windows(*dp, W)
        emi_c, ems_c = _pad_windows(*dc, W)
        pg, pm, _ = _pool_grid(ppn, pps, c * PSEG, PSEG, Lp)
        cg, cm, _ = _pool_grid(pcn, pcs, c * CSEG, CSEG, Lcj)
        # pgrid: [NC, 128, Lp]; masks packed [.., 2] = (mask, bias)
        pgrid = np.ascontiguousarray(pg.reshape(Lp, NC, 128).transpose(1, 2, 0))
        cgrid = np.ascontiguousarray(cg.reshape(Lcj, 128).T)
        pmb = np.stack([pm, np.where(pm > 0, 0.0, -1e38).astype(np.float32)],
                       axis=-1).astype(np.float32).reshape(NC, 128, 2)
        cmb = np.stack([cm, np.where(cm > 0, 0.0, -1e38).astype(np.float32)],
                       axis=-1).astype(np.float32).reshape(128, 2)
        m = dict(shared)
        m.update(
            ns_local=np.ascontiguousarray(ns[lo:lo + NPC], np.float32),
            type_local=np.ascontiguousarray(nt[lo:lo + NPC].reshape(NPC, 1), np.int32),
            emi_p=emi_p, ems_p=ems_p,
            emi_c=emi_c, ems_c=ems_c,
            pgrid=pgrid, pmb=pmb, cgrid=cgrid, cmb=cmb,
            p2c_local=np.ascontiguousarray(
                p2c[c * PSEG:(c + 1) * PSEG].reshape(NC, 128, 1).astype(np.int32)),
        )
        in_maps.append(m)
    return in_maps, W, Lp, Lcj


# ---------------------------------------------------------------- builder

def build(W, Lp, Lcj):
    nc = bacc.Bacc("TRN2", target_bir_lowering=False, debug=False, num_devices=NC)

    # -------- I/O declarations
    inp = {}
    def din(name, shape, dt=F32):
        inp[name] = nc.dram_tensor(name, list(shape), dt, kind="ExternalInput")
        return inp[name]

    din("ns_local", (NPC, D)); din("type_local", (NPC, 1), I32)
    din("ln_g", (1, D)); din("ln_b", (1, D))
    for d in ("p", "c"):
        din(f"wq_{d}", (D, ATT)); din(f"wk_{d}", (D, ATT)); din(f"wv_{d}", (D, ATT))
        din(f"wo_{d}", (ATT, ATT))
        din(f"emi_{d}", (W, 128, 7), I32)
        din(f"ems_{d}", (W, 128, 3))
    din("wdag", (DC, DC)); din("bdag", (1, DC))
    din("embp", (TYPE_CT, TS)); din("wpw", (TS, ATT)); din("wpb", (1, ATT))
    din("embc", (TYPE_CT, TS)); din("wcw", (TS, ATT)); din("wcb", (1, ATT))
    din("pgrid", (NC, 128, Lp), I32); din("pmb", (NC, 128, 2))
    din("cgrid", (128, Lcj), I32); din("cmb", (128, 2))
    din("p2c_local", (NC, 128, 1), I32)

    out_shard = nc.dram_tensor("out_shard", [PSEG, 2 * DC], F32, kind="ExternalOutput")

    NT = (NPC + 127) // 128      # node tiles per core (98)

    with tile.TileContext(nc) as tc:
        with tc.tile_pool(name="dram", bufs=1, space="DRAM") as dram, \
             tc.tile_pool(name="singles", bufs=1) as singles:

            # -------- DRAM internals
            x_local = dram.tile([NPC, D], F32)
            q_p = dram.tile([NPC, ATT], F32)
            q_c = dram.tile([NPC, ATT], F32)
            x_full = dram.tile([N, D], F32, addr_space="Shared")
            agg_p = dram.tile([NPC + 1, ATT], F32)
            agg_c = dram.tile([NPC + 1, ATT], F32)
            pooled_local = dram.tile([NPC, DC], F32)
            pooled_full = dram.tile([N, DC], F32, addr_space="Shared")
            conj_local = dram.tile([CSEG, DC], F32)
            conj_full = dram.tile([PC, DC], F32, addr_space="Shared")
            table = {"p": dram.tile([TYPE_CT, ATT], F32, name="table_p"),
                     "c": dram.tile([TYPE_CT, ATT], F32, name="table_c")}

            # -------- constants / weights in SBUF
            ident = singles.tile([128, 128], F32)
            make_identity(nc, ident[:])
            iota_i = singles.tile([128, 128], I32)
            nc.gpsimd.iota(iota_i[:], pattern=[[1, 128]], base=0, channel_multiplier=0)
            iota_f = singles.tile([128, 128], F32)
            nc.vector.tensor_copy(out=iota_f[:], in_=iota_i[:])
            eps_t = singles.tile([128, 1], F32)
            nc.vector.memset(eps_t[:], 1e-5)

            lng_b = singles.tile([128, D], F32)
            nc.sync.dma_start(out=lng_b[:], in_=inp["ln_g"].ap().to_broadcast([128, D]))
            lnb_b = singles.tile([128, D], F32)
            nc.sync.dma_start(out=lnb_b[:], in_=inp["ln_b"].ap().to_broadcast([128, D]))
            bdag_b = singles.tile([128, DC], F32)
            nc.sync.dma_start(out=bdag_b[:], in_=inp["bdag"].ap().to_broadcast([128, DC]))

            def load_round(name, kc, n):
                """load [kc*128, n] fp32 weight -> f32r chunks [128, kc, n]"""
                t0 = singles.tile([128, kc, n], F32, tag="ldstage")
                nc.sync.dma_start(out=t0[:], in_=inp[name].ap().rearrange(
                    "(c k) n -> k c n", c=kc))
                tr = singles.tile([128, kc, n], F32R, tag=f"ldrr_{name}")
                nc.vector.tensor_copy(out=tr[:], in_=t0[:])
                return tr

            wq_r = {d: load_round(f"wq_{d}", 2, ATT) for d in ("p", "c")}
            wk_r = {d: load_round(f"wk_{d}", 2, ATT) for d in ("p", "c")}
            wv_r = {d: load_round(f"wv_{d}", 2, ATT) for d in ("p", "c")}
            wo_r = {d: load_round(f"wo_{d}", 4, ATT) for d in ("p", "c")}
            wdag_r = load_round("wdag", 6, DC)

            # -------- gate tables: sigmoid(emb @ w + b) -> DRAM [64, ATT]
            with tc.tile_pool(name="gt_sb", bufs=1) as gsb, \
                 tc.tile_pool(name="gt_ps", bufs=1, space="PSUM") as gps:
                for d, emb_n, w_n, b_n in (("p", "embp", "wpw", "wpb"),
                                           ("c", "embc", "wcw", "wcb")):
                    emb_t = gsb.tile([TYPE_CT, TS], F32, tag="emb")
                    nc.sync.dma_start(out=emb_t[:], in_=inp[emb_n].ap())
                    embT_ps = gps.tile([TS, TYPE_CT], F32, tag="embT", space="PSUM")
                    nc.tensor.transpose(out=embT_ps[:], in_=emb_t[:], identity=ident[:TYPE_CT, :TYPE_CT])
                    embT_r = gsb.tile([TS, TYPE_CT], F32R, tag="embTr")
                    nc.vector.tensor_copy(out=embT_r[:], in_=embT_ps[:])
                    w_t = gsb.tile([TS, ATT], F32, tag="gw")
                    nc.sync.dma_start(out=w_t[:], in_=inp[w_n].ap())
                    w_rr = gsb.tile([TS, ATT], F32R, tag="gwr")
                    nc.vector.tensor_copy(out=w_rr[:], in_=w_t[:])
                    g_ps = gps.tile([TYPE_CT, ATT], F32, tag="gps", space="PSUM")
                    nc.tensor.matmul(out=g_ps[:], lhsT=embT_r[:], rhs=w_rr[:],
                                     start=True, stop=True)
                    b_t = gsb.tile([TYPE_CT, ATT], F32, tag="gb")
                    nc.sync.dma_start(out=b_t[:], in_=inp[b_n].ap().to_broadcast([TYPE_CT, ATT]))
                    g_sb = gsb.tile([TYPE_CT, ATT], F32, tag="gsb")
                    nc.vector.tensor_add(out=g_sb[:], in0=g_ps[:], in1=b_t[:])
                    nc.scalar.activation(out=g_sb[:], in_=g_sb[:],
                                         func=mybir.ActivationFunctionType.Sigmoid,
                                         bias=0.0, scale=1.0, alpha=0.0)
                    nc.sync.dma_start(out=table[d][:], in_=g_sb[:])

            # ================ Phase A: LN + Q projections ================
            with tc.tile_pool(name="a_sb", bufs=3) as asb, \
                 tc.tile_pool(name="a_ps", bufs=2, space="PSUM") as aps:
                for it in range(NT):
                    r0 = it * 128
                    sz = min(128, NPC - r0)
                    ns_t = asb.tile([128, D], F32, tag="ns")
                    if sz < 128:
                        nc.vector.memset(ns_t[:], 0.0)
                    nc.sync.dma_start(out=ns_t[:sz], in_=inp["ns_local"].ap()[r0:r0 + sz])
                    stats = asb.tile([128, nc.vector.BN_STATS_DIM], F32, tag="st")
                    nc.vector.bn_stats(out=stats[:], in_=ns_t[:])
                    mv = asb.tile([128, nc.vector.BN_AGGR_DIM], F32, tag="mv")
                    nc.vector.bn_aggr(out=mv[:], in_=stats[:])
                    rstd = asb.tile([128, 1], F32, tag="rs")
                    nc.scalar.activation(out=rstd[:], in_=mv[:, 1:2],
                                         func=mybir.ActivationFunctionType.Sqrt,
                                         bias=eps_t[:], scale=1.0, alpha=0.0)
                    nc.vector.reciprocal(out=rstd[:], in_=rstd[:])
                    x_t = asb.tile([128, D], F32, tag="x")
                    nc.vector.tensor_scalar(
                        out=x_t[:], in0=ns_t[:], scalar1=mv[:, 0:1], scalar2=rstd[:],
                        op0=mybir.AluOpType.subtract, op1=mybir.AluOpType.mult)
                    nc.vector.tensor_mul(out=x_t[:], in0=x_t[:], in1=lng_b[:])
                    nc.vector.tensor_add(out=x_t[:], in0=x_t[:], in1=lnb_b[:])
                    nc.sync.dma_start(out=x_local[r0:r0 + sz], in_=x_t[:sz])

                    xT_ps = aps.tile([128, 2, 128], F32, tag="xT", space="PSUM")
                    for cch in range(2):
                        nc.tensor.transpose(out=xT_ps[:, cch, :],
                                            in_=x_t[:, cch * 128:(cch + 1) * 128],
                                            identity=ident[:])
                    xT_r = asb.tile([128, 2, 128], F32R, tag="xTr")
                    nc.vector.tensor_copy(out=xT_r[:], in_=xT_ps[:])
                    for d, qbuf in (("p", q_p), ("c", q_c)):
                        q_ps = aps.tile([128, ATT], F32, tag="q", space="PSUM")
                        for cch in range(2):
                            nc.tensor.matmul(out=q_ps[:], lhsT=xT_r[:, cch, :],
                                             rhs=wq_r[d][:, cch, :],
                                             start=(cch == 0), stop=(cch == 1))
                        q_sb = asb.tile([128, ATT], F32, tag="qsb")
                        nc.vector.tensor_copy(out=q_sb[:], in_=q_ps[:])
                        nc.sync.dma_start(out=qbuf[r0:r0 + sz], in_=q_sb[:sz])

            # ================ AllGather x ================
            nc.gpsimd.collective_compute(
                "AllGather", mybir.AluOpType.bypass,
                replica_groups=[list(range(NC))],
                ins=[x_local[:].opt()], outs=[x_full[:].opt()])

            # ================ Phase B: edge attention ================
            with tc.tile_pool(name="b_sb", bufs=3) as bsb, \
                 tc.tile_pool(name="b_ps", bufs=2, space="PSUM") as bps, \
                 tc.tile_pool(name="b_ps1", bufs=2, space="PSUM") as bps1:
                for d, qbuf, aggbuf in (("p", q_p, agg_p), ("c", q_c, agg_c)):
                    for w in range(W):
                        agg_ps = bps1.tile([128, ATT], F32, tag="agg", space="PSUM")
                        den_ps = bps1.tile([128, H], F32, tag="den", space="PSUM")
                        meta = bsb.tile([128, 7], I32, tag="meta")
                        nc.sync.dma_start(out=meta[:], in_=inp[f"emi_{d}"].ap()[w])
                        segf = bsb.tile([128, 3], F32, tag="segf")
                        nc.sync.dma_start(out=segf[:], in_=inp[f"ems_{d}"].ap()[w])
                        for t3 in range(3):
                            xt = bsb.tile([128, D], F32, tag="xt")
                            nc.gpsimd.indirect_dma_start(
                                out=xt[:], out_offset=None, in_=x_full[:],
                                in_offset=bass.IndirectOffsetOnAxis(
                                    ap=meta[:, 2 * t3:2 * t3 + 1], axis=0))
                            qs = bsb.tile([128, ATT], F32, tag="qs")
                            nc.gpsimd.indirect_dma_start(
                                out=qs[:], out_offset=None, in_=qbuf[:],
                                in_offset=bass.IndirectOffsetOnAxis(
                                    ap=meta[:, 2 * t3 + 1:2 * t3 + 2], axis=0))
                            xtT_ps = bps.tile([128, 2, 128], F32, tag="xtT", space="PSUM")
                            for cch in range(2):
                                nc.tensor.transpose(out=xtT_ps[:, cch, :],
                                                    in_=xt[:, cch * 128:(cch + 1) * 128],
                                                    identity=ident[:])
                            xtT_r = bsb.tile([128, 2, 128], F32R, tag="xtTr")
                            nc.vector.tensor_copy(out=xtT_r[:], in_=xtT_ps[:])
                            k_ps = bps.tile([128, ATT], F32, tag="kv", space="PSUM")
                            for cch in range(2):
                                nc.tensor.matmul(out=k_ps[:], lhsT=xtT_r[:, cch, :],
                                                 rhs=wk_r[d][:, cch, :],
                                                 start=(cch == 0), stop=(cch == 1))
                            v_ps = bps.tile([128, ATT], F32, tag="kv", space="PSUM")
                            for cch in range(2):
                                nc.tensor.matmul(out=v_ps[:], lhsT=xtT_r[:, cch, :],
                                                 rhs=wv_r[d][:, cch, :],
                                                 start=(cch == 0), stop=(cch == 1))
                            tq = bsb.tile([128, ATT], F32, tag="tq")
                            nc.vector.tensor_mul(out=tq[:], in0=qs[:], in1=k_ps[:])
                            sc = bsb.tile([128, H], F32, tag="sc")
                            nc.vector.tensor_reduce(
                                out=sc[:], in_=tq[:].rearrange("p (h k) -> p h k", h=H),
                                axis=mybir.AxisListType.X, op=mybir.AluOpType.add)
                            ex = bsb.tile([128, H], F32, tag="ex")
                            nc.scalar.activation(out=ex[:], in_=sc[:],
                                                 func=mybir.ActivationFunctionType.Exp,
                                                 bias=0.0, scale=0.125, alpha=0.0)
                            S = bsb.tile([128, 128], F32R, tag="S")
                            nc.vector.tensor_tensor(
                                out=S[:], in0=iota_f[:],
                                in1=segf[:, t3:t3 + 1].to_broadcast([128, 128]),
                                op=mybir.AluOpType.is_equal)
                            wv_t = bsb.tile([128, ATT + H], F32R, tag="wv")
                            nc.vector.tensor_tensor(
                                out=wv_t[:, :ATT].rearrange("p (h k) -> p h k", h=H),
                                in0=v_ps[:].rearrange("p (h k) -> p h k", h=H),
                                in1=ex[:, :, None].to_broadcast([128, H, HD]),
                                op=mybir.AluOpType.mult)
                            nc.vector.tensor_copy(out=wv_t[:, ATT:], in_=ex[:])
                            nc.tensor.matmul(out=agg_ps[:], lhsT=S[:], rhs=wv_t[:, :ATT],
                                             start=(t3 == 0), stop=(t3 == 2))
                            nc.tensor.matmul(out=den_ps[:], lhsT=S[:], rhs=wv_t[:, ATT:],
                                             start=(t3 == 0), stop=(t3 == 2))
                        # window close
                        rec = bsb.tile([128, H], F32, tag="rec")
                        nc.vector.tensor_scalar(out=rec[:], in0=den_ps[:], scalar1=1e-30,
                                                scalar2=None, op0=mybir.AluOpType.add)
                        nc.vector.reciprocal(out=rec[:], in_=rec[:])
                        aggn = bsb.tile([128, ATT], F32, tag="aggn")
                        nc.vector.tensor_tensor(
                            out=aggn[:].rearrange("p (h k) -> p h k", h=H),
                            in0=agg_ps[:].rearrange("p (h k) -> p h k", h=H),
                            in1=rec[:, :, None].to_broadcast([128, H, HD]),
                            op=mybir.AluOpType.mult)
                        nc.gpsimd.indirect_dma_start(
                            out=aggbuf[:],
                            out_offset=bass.IndirectOffsetOnAxis(ap=meta[:, 6:7], axis=0),
                            in_=aggn[:], in_offset=None)

            # ================ Phase C: Wo + gates + W_dag + tanh ================
            with tc.tile_pool(name="c_sb", bufs=2) as csb, \
                 tc.tile_pool(name="c_ps", bufs=2, space="PSUM") as cps, \
                 tc.tile_pool(name="c_ps1", bufs=1, space="PSUM") as cps1:
                for it in range(NT):
                    r0 = it * 128
                    sz = min(128, NPC - r0)
                    comb_ps = {}
                    for d, aggbuf in (("p", agg_p), ("c", agg_c)):
                        a_t = csb.tile([128, ATT], F32, tag="a")
                        if sz < 128:
                            nc.vector.memset(a_t[:], 0.0)
                        nc.sync.dma_start(out=a_t[:sz], in_=aggbuf[r0:r0 + sz])
                        aT_ps = cps.tile([128, 4, 128], F32, tag="aT", space="PSUM")
                        for cch in range(4):
                            nc.tensor.transpose(out=aT_ps[:, cch, :],
                                                in_=a_t[:, cch * 128:(cch + 1) * 128],
                                                identity=ident[:])
                        aT_r = csb.tile([128, 4, 128], F32R, tag="aTr")
                        nc.vector.tensor_copy(out=aT_r[:], in_=aT_ps[:])
                        cps_d = cps.tile([128, ATT], F32, tag="comb", space="PSUM")
                        for cch in range(4):
                            nc.tensor.matmul(out=cps_d[:], lhsT=aT_r[:, cch, :],
                                             rhs=wo_r[d][:, cch, :],
                                             start=(cch == 0), stop=(cch == 3))
                        comb_ps[d] = cps_d
                    ty_t = csb.tile([128, 1], I32, tag="ty")
                    if sz < 128:
                        nc.vector.memset(ty_t[:], 0)
                    nc.sync.dma_start(out=ty_t[:sz], in_=inp["type_local"].ap()[r0:r0 + sz])
                    comb = csb.tile([128, ATT], F32, tag="cmb")
                    first = True
                    for d in ("p", "c"):
                        g_t = csb.tile([128, ATT], F32, tag="g")
                        nc.gpsimd.indirect_dma_start(
                            out=g_t[:], out_offset=None, in_=table[d][:],
                            in_offset=bass.IndirectOffsetOnAxis(ap=ty_t[:, 0:1], axis=0))
                        m_t = csb.tile([128, ATT], F32, tag="m")
                        nc.vector.tensor_mul(out=m_t[:], in0=g_t[:], in1=comb_ps[d][:])
                        if first:
                            nc.vector.tensor_copy(out=comb[:], in_=m_t[:])
                            first = False
                        else:
                            nc.vector.tensor_add(out=comb[:], in0=comb[:], in1=m_t[:])
                    x_t = csb.tile([128, D], F32, tag="x2")
                    if sz < 128:
                        nc.vector.memset(x_t[:], 0.0)
                    nc.sync.dma_start(out=x_t[:sz], in_=x_local[r0:r0 + sz])
                    hT_ps = cps1.tile([128, 6, 128], F32, tag="hT", space="PSUM")
                    for cch in range(2):
                        nc.tensor.transpose(out=hT_ps[:, cch, :],
                                            in_=x_t[:, cch * 128:(cch + 1) * 128],
                                            identity=ident[:])
                    for cch in range(4):
                        nc.tensor.transpose(out=hT_ps[:, 2 + cch, :],
                                            in_=comb[:, cch * 128:(cch + 1) * 128],
                                            identity=ident[:])
                    hT_r = csb.tile([128, 6, 128], F32R, tag="hTr")
                    nc.vector.tensor_copy(out=hT_r[:], in_=hT_ps[:])
                    pooled = csb.tile([128, DC], F32, tag="pl")
                    for half in range(2):
                        p_ps = cps.tile([128, DC // 2], F32, tag="pp", space="PSUM")
                        for cch in range(6):
                            nc.tensor.matmul(
                                out=p_ps[:], lhsT=hT_r[:, cch, :],
                                rhs=wdag_r[:, cch, half * (DC // 2):(half + 1) * (DC // 2)],
                                start=(cch == 0), stop=(cch == 5))
                        nc.vector.tensor_add(
                            out=pooled[:, half * (DC // 2):(half + 1) * (DC // 2)],
                            in0=p_ps[:], in1=bdag_b[:, half * (DC // 2):(half + 1) * (DC // 2)])
                    nc.scalar.activation(out=pooled[:], in_=pooled[:],
                                         func=mybir.ActivationFunctionType.Tanh,
                                         bias=0.0, scale=1.0, alpha=0.0)
                    nc.sync.dma_start(out=pooled_local[r0:r0 + sz], in_=pooled[:sz])

            # ================ AllGather pooled ================
            nc.gpsimd.collective_compute(
                "AllGather", mybir.AluOpType.bypass,
                replica_groups=[list(range(NC))],
                ins=[pooled_local[:].opt()], outs=[pooled_full[:].opt()])

            # ================ Phase D: pool segment-max ================
            with tc.tile_pool(name="d_sb", bufs=2) as dsb:
                def seg_max(grid_src, mb_src, L, out_ap):
                    gi = dsb.tile([128, L], I32, tag="gi")
                    nc.sync.dma_start(out=gi[:], in_=grid_src)
                    mb = dsb.tile([128, 2], F32, tag="mb")
                    nc.sync.dma_start(out=mb[:], in_=mb_src)
                    acc = dsb.tile([128, DC], F32, tag="acc")
                    for l in range(L):
                        g_t = dsb.tile([128, DC], F32, tag="gt")
                        nc.gpsimd.indirect_dma_start(
                            out=g_t[:], out_offset=None, in_=pooled_full[:],
                            in_offset=bass.IndirectOffsetOnAxis(ap=gi[:, l:l + 1], axis=0))
                        if l == 0:
                            nc.vector.tensor_copy(out=acc[:], in_=g_t[:])
                        else:
                            nc.vector.tensor_tensor(out=acc[:], in0=acc[:], in1=g_t[:],
                                                    op=mybir.AluOpType.max)
                    nc.vector.tensor_scalar(out=acc[:], in0=acc[:], scalar1=mb[:, 0:1],
                                            scalar2=mb[:, 1:2],
                                            op0=mybir.AluOpType.mult,
                                            op1=mybir.AluOpType.add)
                    nc.sync.dma_start(out=out_ap, in_=acc[:])

                for ch in range(NC):
                    seg_max(inp["pgrid"].ap()[ch], inp["pmb"].ap()[ch], Lp,
                            out_shard.ap()[ch * 128:(ch + 1) * 128, 0:DC])
                seg_max(inp["cgrid"].ap(), inp["cmb"].ap(), Lcj, conj_local[:])

            nc.gpsimd.collective_compute(
                "AllGather", mybir.AluOpType.bypass,
                replica_groups=[list(range(NC))],
                ins=[conj_local[:].opt()], outs=[conj_full[:].opt()])

            with tc.tile_pool(name="e_sb", bufs=2) as esb:
                for ch in range(NC):
                    pi = esb.tile([128, 1], I32, tag="pi")
                    nc.sync.dma_start(out=pi[:], in_=inp["p2c_local"].ap()[ch])
                    cr = esb.tile([128, DC], F32, tag="cr")
                    nc.gpsimd.indirect_dma_start(
                        out=cr[:], out_offset=None, in_=conj_full[:],
                        in_offset=bass.IndirectOffsetOnAxis(ap=pi[:, 0:1], axis=0))
                    nc.sync.dma_start(
                        out=out_shard.ap()[ch * 128:(ch + 1) * 128, DC:2 * DC],
                        in_=cr[:])

    return nc


# ---------------------------------------------------------------- entry

_CACHE = {}

def _get_compiled(W, Lp, Lcj):
    key = (W, Lp, Lcj)
    if key not in _CACHE:
        nc = build(W, Lp, Lcj)
        nc.compile()
        _CACHE[key] = nc
    return _CACHE[key]


def run(inputs, trace=False):
    in_maps, W, Lp, Lcj = preprocess(inputs)
    nc = _get_compiled(W, Lp, Lcj)
    res = bass_utils.run_bass_kernel_spmd(
        nc, in_maps, core_ids=list(range(NC)), trace=trace)
    out = np.concatenate([r["out_shard"] for r in res.results], axis=0)
    return out, res


def kernel(**inputs) -> np.ndarray:
    out, _ = run(inputs, trace=False)
    return out
